# revision 60
# baseline (speedup 1.0000x reference)
"""NT-Xent loss kernel for Trainium2, distributed across 8 NeuronCores.

Strategy: each core receives the full [8192, 128] input, rotated by 1024*c
rows so the kernel is pure SPMD — every core computes the row sums of
exp(sim/T) for the *first* 1024 local rows against all 8192 columns.

Per core:
  - load x (rows-on-partitions layout, 64 chunks of [128, 128])
  - norms via bn_stats (one DVE pass) + tiny fixup ops
  - r = SQK * s^(-1/2) via exp(-0.5 * ln(s) + ln(SQK)); normalizing by r
    pre-scales the matrix so PSUM sim values are sim*K16 with
    K16 = 256/ln2 — directly the bf16-exponent scale used by the
    Schraudolph trick below.
  - normalize rows -> bf16 (DVE), PE-transpose chunks -> xnT, ACT copies
  - main loop, 32 tiles of [128 rows x 2048 cols], two exp lanes:
      ACT lane: exp via activation (scale=2/K16) with accum_out row sums
      DVE lane: Schraudolph bf16 exp: i16 = round(PSUM + BADD) reinterpreted
        as bf16 IS exp(2*sim) to ~2%; then a tensor_scalar pass with
        accum_out produces the row sums. Calibrated so the row-sum bias
        is ~1e-3 (C=6).
  - row totals - e^2 (diag), ln + accumulate, minus (2/K16)*possum,
    partition-reduce via ones-matmul -> scalar partial loss
Host sums the 8 partial scalars.

Hardware constraint honored throughout: a Matmult instruction can carry
only ONE sync-wait; a generalized strip pass removes waits that are
transitively implied by each engine's in-order queue.
"""

import math

import numpy as np

import concourse.bass as bass
import concourse.tile as tile
from concourse import mybir
from concourse.bass_utils import run_bass_kernel_spmd
from concourse.masks import make_identity

N2 = 8192          # total rows (2N)
D = 128            # feature dim
NCORES = 8
RPC = N2 // NCORES  # rows per core = 1024
NCHUNK = N2 // 128  # 64 chunks of 128 rows
F32 = mybir.dt.float32
BF16 = mybir.dt.bfloat16
I16 = mybir.dt.int16
AF = mybir.ActivationFunctionType
ALU = mybir.AluOpType
E2 = float(np.exp(2.0, dtype=np.float64))  # diag term exp(sim_ii / T), T=0.5

K16 = 256.0 / math.log(2.0)        # PSUM scale: psum = sim * K16
SQK = math.sqrt(K16)               # per-vector pre-scale
BADD = 16256.0 - 6.0               # Schraudolph bias (C=6, round-to-nearest)
# tiles whose exp goes through the DVE Schraudolph lane (j = 8*g + m).
# Chosen so every matmul carries <=1 sync wait after the strip pass: the
# PSUM slot preceding each col-group start (j=8k, slot A, prior reader
# j=8k-2) must have an ACT reader so the group's new xnT-copy wait merges
# with the slot wait on the ACT semaphore.
DVE_TILES = frozenset()  # Schraudolph lane disabled: DVE-from-PSUM lowering
                         # proved unreliable on HW (see transcript)
EMIT_XPROBE = False  # debug toggle


DEBUG_OUTS = False  # emit extra HBM dumps of intermediates


def _emit(tc: tile.TileContext, ctx, out_ap: bass.AP, x_ap: bass.AP,
          dbg_aps: dict | None = None):
    nc = tc.nc

    big = ctx.enter_context(tc.tile_pool(name="big", bufs=1))
    esc = ctx.enter_context(tc.tile_pool(name="esc", bufs=3))
    dsc = ctx.enter_context(tc.tile_pool(name="dsc", bufs=2))
    small = ctx.enter_context(tc.tile_pool(name="small", bufs=1))

    # one tile per DMA group: keeps each consumer waiting on a single DMA sem
    x_g = [
        big.tile([128, 8, 128], F32, tag=f"x{g}", name=f"x_{g}") for g in range(8)
    ]
    xb = big.tile([128, NCHUNK, 128], BF16, tag="xb")     # normalized*SQK, bf16
    # transposed normalized matrix, split into 4 tiles (finer matmul deps)
    xnT = [
        big.tile([128, 2048], BF16, tag=f"xnT{t}", name=f"xnT_{t}")
        for t in range(4)
    ]
    xsq_g = [
        big.tile([128, 8, 128], F32, tag=f"xsq{g}", name=f"xsq_{g}")
        for g in range(8)
    ]

    s = small.tile([128, NCHUNK], F32)     # squared norms (row 128c+p at [p, c])
    ls = small.tile([128, NCHUNK], F32)
    r = small.tile([128, NCHUNK], F32)     # SQK/norm
    r_dve = small.tile([128, NCHUNK], F32)  # DVE-local copy

    rs_a = small.tile([128, 32], F32)      # ACT-lane accum slots (g, m)
    rs_d = small.tile([128, 32], F32)      # DVE-lane accum slots
    rt_a = small.tile([128, 8], F32)       # row totals per Mtile
    rt_d = small.tile([128, 8], F32)
    rt = small.tile([128, 8], F32)
    lg = small.tile([128, 8], F32)
    logsum = small.tile([128, 1], F32)
    possum = small.tile([128, 1], F32)
    fin = small.tile([128, 1], F32)
    fin2 = small.tile([128, 1], F32)       # ACT-written copy (matmul 1-wait rule)
    ones = small.tile([128, 1], F32)       # ACT-written
    ident = small.tile([128, 128], BF16)
    dummy_bf = small.tile([128, 1], BF16)  # ACT-written transpose lead input
    iprobe = small.tile([1, 1], BF16)      # DVE probe of ident (Pool->DVE edge)
    fin_sb = small.tile([1, 1], F32)
    pos_scr = small.tile([128, RPC], BF16)
    negE2 = small.tile([128, 1], F32)



    make_identity(nc, ident)
    # DVE probe-read of ident FIRST: all later DVE ops (and, through them,
    # ACT's constant writes) transitively imply the identity is built
    nc.vector.tensor_copy(iprobe, ident[0:1, 0:1])
    nc.vector.memset(negE2, -E2)
    nc.vector.memset(rs_a, 0.0)
    nc.vector.memset(rs_d, 0.0)
    # ones written by ACT so the final matmul waits on ACT only
    nc.scalar.activation(out=ones, in_=negE2, func=AF.Copy, bias=1.0, scale=0.0)
    nc.scalar.activation(out=dummy_bf, in_=negE2, func=AF.Copy)

    x_src = x_ap.rearrange("(c p) d -> p c d", p=128)

    # ---- fused prep + main loop, one iteration per col-group t ----
    # Each iteration preps chunk-groups 2t,2t+1 (load, norms, normalize),
    # DMA-xbar-transposes them into xnT[t], probes them on ACT (so matmuls
    # can single-wait on ACT), then runs the 8 Mtile matmuls + exps of
    # column group t. ACT's in-order queue therefore starts the first exps
    # as soon as the first two chunk-groups are ready instead of after the
    # whole prep.
    ps = ctx.enter_context(tc.tile_pool(name="ps", bufs=2, space="PSUM"))
    # kick off every input load up front: the DMA queues are idle early
    for gg in range(8):
        sl = slice(8 * gg, 8 * gg + 8)
        nc.sync.dma_start(out=x_g[gg][:, :, :], in_=x_src[:, sl, :])
    for t in range(4):
        # PE-transposes go INTO the first main PSUM tile of this group
        # (bitcast to bf16 halves), then DVE-copies move 512-col slices to
        # xnT in SBUF as soon as their 4 chunks are transposed. The m=0
        # matmuls then overwrite the same tile, so the chain needs no
        # extra PSUM and every instruction carries one wait after the
        # strip: the lead dummy transpose takes the slot-WAR (ACT) merged
        # with its ACT-written input dep; real transposes wait only on DVE
        # normalize; copies wait on PE; matmuls wait on DVE.
        pm0 = ps.tile([128, 2048], F32, tag="pm", name=f"pm_0_{t}")
        pm0_bf = pm0.bitcast(BF16)
        nc.tensor.transpose(pm0_bf[0:1, 0:128], dummy_bf, ident)
        for gg in (2 * t, 2 * t + 1):
            sl = slice(8 * gg, 8 * gg + 8)
            nc.vector.tensor_mul(
                xsq_g[gg][:, :, :], x_g[gg][:, :, :], x_g[gg][:, :, :]
            )
            nc.vector.tensor_reduce(
                out=s[:, sl],
                in_=xsq_g[gg][:, :, :],
                axis=mybir.AxisListType.X,
                op=ALU.add,
            )
            # r = s^-1/2 via exp(-0.5*ln(s)); the SQK pre-scale is folded
            # into the DVE-local copy below
            nc.scalar.activation(out=ls[:, sl], in_=s[:, sl], func=AF.Ln)
            nc.scalar.activation(
                out=r[:, sl], in_=ls[:, sl], func=AF.Exp, scale=-0.5
            )
            nc.vector.tensor_scalar_mul(
                out=r_dve[:, sl], in0=r[:, sl], scalar1=SQK
            )
            for c in range(8 * gg, 8 * gg + 8):
                nc.vector.tensor_scalar_mul(
                    out=xb[:, c, :],
                    in0=x_g[c // 8][:, c % 8, :],
                    scalar1=r_dve[:, c : c + 1],
                )
            h = gg % 2  # chunk half within this column group
            for k in range(8 * h, 8 * h + 8):
                ch = 16 * t + k
                nc.tensor.transpose(
                    pm0_bf[:, 128 * k : 128 * (k + 1)], xb[:, ch, :], ident
                )
            for q in range(2 * h, 2 * h + 2):
                nc.vector.tensor_copy(
                    xnT[t][:, 512 * q : 512 * (q + 1)],
                    pm0_bf[:, 512 * q : 512 * (q + 1)],
                )
        for m in range(8):
            j = 8 * t + m
            pm = pm0 if m == 0 else ps.tile(
                [128, 2048], F32, tag="pm", name=f"pm_{m}_{t}"
            )
            lhsT = xnT[0][:, 128 * m : 128 * (m + 1)]
            for k in range(4):
                nc.tensor.matmul(
                    pm[:, 512 * k : 512 * (k + 1)],
                    lhsT=lhsT,
                    rhs=xnT[t][:, 512 * k : 512 * (k + 1)],
                    start=True,
                    stop=True,
                )
            e_t = esc.tile([128, 2048], BF16, tag="e", name=f"e_{m}_{t}")
            nc.scalar.activation(
                out=e_t[:, :],
                in_=pm[:, :],
                func=AF.Exp,
                scale=2.0 / K16,
                accum_out=rs_a[:, j : j + 1],
            )

    # ---- positive-pair term: sum over my rows of sim(i, i+N)*K16 ----
    # local pos column of local row i is always i + 4096 (rotation invariant)
    nc.vector.tensor_mul(pos_scr, xnT[0][:, 0:RPC], xnT[2][:, 0:RPC])
    nc.vector.tensor_reduce(
        out=possum, in_=pos_scr, axis=mybir.AxisListType.X, op=ALU.add
    )

    # ---- finals: gather raw accumulators, finish (ln + sums) on host ----
    # One DVE gather makes the output DMA single-wait (DVE), and the
    # ln/partition-reduction tail (which used to cost ~10us of serial
    # ACT/DVE/PE ping-pong) moves to the host, where it is trivial.
    gather = small.tile([128, 65], F32, name="gather")
    nc.vector.tensor_copy(gather[:, 0:32], rs_a)
    nc.vector.tensor_copy(gather[:, 32:64], rs_d)
    nc.vector.tensor_copy(gather[:, 64:65], possum)
    # SWDGE for the output write: the HWDGE direct-2D encoding only
    # carries one sync wait and this DMA needs a data wait on DVE
    nc.gpsimd.dma_start(out=out_ap, in_=gather)


def _strip_waits(nc):
    """Knowledge-propagating transitive reduction of sync waits.

    Model: every sem update is attributed to its instruction in program
    order; waits gate instruction ISSUE, and each queue issues in order.
    So when instruction i on engine E waits (S >= v), E afterwards "knows"
    everything the instruction that drove S to v knew at its completion.
    A wait is redundant iff E's accumulated knowledge already implies it.

    Engine self-waits on the IMMEDIATELY PRECEDING instruction are kept
    (completion can lag issue on pipelined engines); deeper self-waits are
    dropped. Matmult/TensorScalarPtr carry only ONE wait slot in their
    encodings, so this reduction is what makes them encodable at all.
    """
    eng_prefix = {
        mybir.EngineType.PE: "PE_",
        mybir.EngineType.Activation: "Activation_",
        mybir.EngineType.DVE: "DVE_",
        mybir.EngineType.Pool: "Pool_",
        mybir.EngineType.SP: "SP_",
    }
    skip_types = {"InstEventSemaphore", "InstNotify"}

    know: dict = {e: {} for e in eng_prefix}   # engine -> {sem: floor}
    # sem -> list of (count_after_update, knowledge snapshot dict)
    snaps: dict = {}
    sem_count: dict = {}

    def merge(dst, src):
        for k, v in src.items():
            if dst.get(k, -1) < v:
                dst[k] = v

    def snap_at(sem, v):
        lst = snaps.get(sem)
        if not lst:
            return None
        for cnt, kn in lst:
            if cnt >= v:
                return kn
        return None

    for bb in nc.main_func.blocks:
        for ins in bb.instructions:
            si = ins.sync_info
            tname = type(ins).__name__
            eng = getattr(ins, "engine", None)
            if si is None or tname in skip_types or eng not in eng_prefix:
                continue
            if tname == "InstDrain":
                w = list(si.on_wait)
                if len(w) > 1 and any(
                    (x.ant_name or "").startswith("DMASW") for x in w
                ):
                    si.on_wait = [
                        x for x in w if (x.ant_name or "").startswith("DMASW")
                    ]
                continue
            pfx = eng_prefix[eng]
            K = know[eng]
            # pass 1: classify waits, gather snapshots
            cands = []
            w2 = []
            for x in list(si.on_wait):
                name = x.ant_name or ""
                try:
                    val = int(x.wait_value)
                except (TypeError, ValueError):
                    w2.append(x)
                    continue
                if name.startswith(pfx):
                    # self-wait: queue order implies it, except possibly for
                    # the immediately preceding instruction on a pipelined
                    # engine. Keep that case (except matmuls: PE serializes)
                    # -- but still let another wait's completion-snapshot
                    # prove it redundant (pass 2).
                    if (
                        val >= sem_count.get(name, 0)
                        and tname != "InstMatmult"
                    ):
                        cands.append((x, name, val, None, True))
                    continue
                cands.append((x, name, val, snap_at(name, val), False))
            # pass 2: drop waits implied by prior knowledge or by the
            # knowledge carried by the OTHER waits' snapshots. Self-waits
            # may only be proven by another wait's completion-snapshot
            # (K tracks issue-order for the own sem, not completion).
            for i, (x, name, val, sn, selfw) in enumerate(cands):
                implied = (not selfw) and K.get(name, -1) >= val
                if not implied:
                    for j2, (_, n2, v2, sn2, _s2) in enumerate(cands):
                        if j2 == i:
                            continue
                        if not selfw and n2 == name and v2 >= val:
                            implied = i > j2  # keep one of equal waits
                        if sn2 and sn2.get(name, -1) >= val:
                            implied = True
                        if implied:
                            break
                if not implied:
                    w2.append(x)
                K[name] = max(K.get(name, 0), val)
                if sn:
                    merge(K, sn)
            si.on_wait = w2
            # record this instruction's updates with a knowledge snapshot
            for u in list(si.on_update or []):
                name = u.ant_name or ""
                try:
                    uval = int(u.update_value)
                except (TypeError, ValueError):
                    uval = 1
                c = sem_count.get(name, 0) + uval
                sem_count[name] = c
                K[name] = max(K.get(name, 0), c)
                snaps.setdefault(name, []).append((c, dict(K)))


def _build(strip: bool = True):
    from contextlib import ExitStack

    nc = bass.Bass("TRN2", debug=False, num_devices=NCORES)
    x_in = nc.dram_tensor("x", [N2, D], F32, kind="ExternalInput")
    out = nc.dram_tensor("out", [128, 65], F32, kind="ExternalOutput")
    dbg_aps = None
    if DEBUG_OUTS:
        shapes = {"s": [128, 64], "r": [128, 64], "rs_a": [128, 32],
                  "rs_d": [128, 32], "rt": [128, 8], "lg": [128, 8],
                  "xb": [128, 128], "xnt": [128, 128]}
        dbg_aps = {
            k: nc.dram_tensor(f"dbg_{k}", v, F32 if k not in ("xb", "xnt")
                              else mybir.dt.bfloat16,
                              kind="ExternalOutput").ap()
            for k, v in shapes.items()
        }
    with tile.TileContext(nc) as tc:
        with ExitStack() as ctx:
            _emit(tc, ctx, out.ap(), x_in.ap(), dbg_aps)
    if strip:
        # CoreSim's race detector models engines as concurrent and would
        # flag the removed (redundant-on-HW) waits; validate numerics
        # with strip=False, ship with strip=True.
        _strip_waits(nc)
    return nc


_NC_CACHE = None


def _get_nc():
    global _NC_CACHE
    if _NC_CACHE is None:
        _NC_CACHE = _build()
    return _NC_CACHE


def kernel(**inputs) -> np.ndarray:
    x = np.ascontiguousarray(
        np.asarray(inputs["projected_vectors"]), dtype=np.float32
    )
    assert x.shape == (N2, D)
    nc = _get_nc()
    in_maps = [
        {"x": np.ascontiguousarray(np.roll(x, -RPC * c, axis=0))}
        for c in range(NCORES)
    ]
    res = run_bass_kernel_spmd(nc, in_maps, core_ids=list(range(NCORES)))
    return finish(res.results)


def finish(results) -> np.ndarray:
    """Host-side finale: per-core ln(rowsum - e^2) sums + positive terms."""
    total = 0.0
    for rmap in results:
        g = np.asarray(rmap["out"], dtype=np.float64)
        rs = g[:, 0:32] + g[:, 32:64]           # [p, 8t+m] partial row sums
        rowsum = rs.reshape(128, 4, 8).sum(axis=1)   # [p, m]
        possum = g[:, 64].sum()                 # sum over partitions (dims)
        total += np.sum(np.log(rowsum - E2)) - (2.0 / K16) * possum
    return np.asarray(total, dtype=np.float32)


if __name__ == "__main__":
    xt = np.random.randn(N2, D).astype(np.float32)
    print(kernel(projected_vectors=xt))


# revision 66
# speedup vs baseline: 1.0388x; 1.0388x over previous
"""NT-Xent loss kernel for Trainium2, distributed across 8 NeuronCores.

Strategy: each core receives the full [8192, 128] input, rotated by 1024*c
rows so the kernel is pure SPMD — every core computes the row sums of
exp(sim/T) for the *first* 1024 local rows against all 8192 columns.

Per core:
  - load x (rows-on-partitions layout, 64 chunks of [128, 128])
  - norms via bn_stats (one DVE pass) + tiny fixup ops
  - r = SQK * s^(-1/2) via exp(-0.5 * ln(s) + ln(SQK)); normalizing by r
    pre-scales the matrix so PSUM sim values are sim*K16 with
    K16 = 256/ln2 — directly the bf16-exponent scale used by the
    Schraudolph trick below.
  - normalize rows -> bf16 (DVE), PE-transpose chunks -> xnT, ACT copies
  - main loop, 32 tiles of [128 rows x 2048 cols], two exp lanes:
      ACT lane: exp via activation (scale=2/K16) with accum_out row sums
      DVE lane: Schraudolph bf16 exp: i16 = round(PSUM + BADD) reinterpreted
        as bf16 IS exp(2*sim) to ~2%; then a tensor_scalar pass with
        accum_out produces the row sums. Calibrated so the row-sum bias
        is ~1e-3 (C=6).
  - row totals - e^2 (diag), ln + accumulate, minus (2/K16)*possum,
    partition-reduce via ones-matmul -> scalar partial loss
Host sums the 8 partial scalars.

Hardware constraint honored throughout: a Matmult instruction can carry
only ONE sync-wait; a generalized strip pass removes waits that are
transitively implied by each engine's in-order queue.
"""

import math

import numpy as np

import concourse.bass as bass
import concourse.tile as tile
from concourse import mybir
from concourse.bass_utils import run_bass_kernel_spmd
from concourse.masks import make_identity

N2 = 8192          # total rows (2N)
D = 128            # feature dim
NCORES = 8
RPC = N2 // NCORES  # rows per core = 1024
NCHUNK = N2 // 128  # 64 chunks of 128 rows
F32 = mybir.dt.float32
BF16 = mybir.dt.bfloat16
I16 = mybir.dt.int16
AF = mybir.ActivationFunctionType
ALU = mybir.AluOpType
E2 = float(np.exp(2.0, dtype=np.float64))  # diag term exp(sim_ii / T), T=0.5

K16 = 256.0 / math.log(2.0)        # PSUM scale: psum = sim * K16
SQK = math.sqrt(K16)               # per-vector pre-scale
BADD = 16256.0 - 6.0               # Schraudolph bias (C=6, round-to-nearest)
# tiles whose exp goes through the DVE Schraudolph lane (j = 8*g + m).
# Chosen so every matmul carries <=1 sync wait after the strip pass: the
# PSUM slot preceding each col-group start (j=8k, slot A, prior reader
# j=8k-2) must have an ACT reader so the group's new xnT-copy wait merges
# with the slot wait on the ACT semaphore.
DVE_TILES = frozenset()  # Schraudolph lane disabled: DVE-from-PSUM lowering
                         # proved unreliable on HW (see transcript)
EMIT_XPROBE = False  # debug toggle


DEBUG_OUTS = False  # emit extra HBM dumps of intermediates


def _emit(tc: tile.TileContext, ctx, out_ap: bass.AP, x_ap: bass.AP,
          dbg_aps: dict | None = None):
    nc = tc.nc

    big = ctx.enter_context(tc.tile_pool(name="big", bufs=1))
    esc = ctx.enter_context(tc.tile_pool(name="esc", bufs=3))
    dsc = ctx.enter_context(tc.tile_pool(name="dsc", bufs=2))
    small = ctx.enter_context(tc.tile_pool(name="small", bufs=1))

    # one tile per DMA group: keeps each consumer waiting on a single DMA sem
    x_g = [
        big.tile([128, 8, 128], F32, tag=f"x{g}", name=f"x_{g}") for g in range(8)
    ]
    xb = big.tile([128, NCHUNK, 128], BF16, tag="xb")     # normalized*SQK, bf16
    # transposed normalized matrix, split into 4 tiles (finer matmul deps)
    xnT = [
        big.tile([128, 2048], BF16, tag=f"xnT{t}", name=f"xnT_{t}")
        for t in range(4)
    ]
    xsq_g = [
        big.tile([128, 8, 128], F32, tag=f"xsq{g}", name=f"xsq_{g}")
        for g in range(8)
    ]

    s = small.tile([128, NCHUNK], F32)     # squared norms (row 128c+p at [p, c])
    ls = small.tile([128, NCHUNK], F32)
    r = small.tile([128, NCHUNK], F32)     # SQK/norm
    r_dve = small.tile([128, NCHUNK], F32)  # DVE-local copy

    rs_a = small.tile([128, 32], F32)      # ACT-lane accum slots (g, m)
    rs_d = small.tile([128, 32], F32)      # DVE-lane accum slots
    rt_a = small.tile([128, 8], F32)       # row totals per Mtile
    rt_d = small.tile([128, 8], F32)
    rt = small.tile([128, 8], F32)
    lg = small.tile([128, 8], F32)
    logsum = small.tile([128, 1], F32)
    possum = small.tile([128, 1], F32)
    fin = small.tile([128, 1], F32)
    fin2 = small.tile([128, 1], F32)       # ACT-written copy (matmul 1-wait rule)
    ones = small.tile([128, 1], F32)       # ACT-written
    ident = small.tile([128, 128], BF16)
    dummy_bf = small.tile([128, 1], BF16)  # ACT-written transpose lead input
    iprobe = small.tile([1, 1], BF16)      # DVE probe of ident (Pool->DVE edge)
    fin_sb = small.tile([1, 1], F32)
    pos_scr = small.tile([128, RPC], BF16)
    negE2 = small.tile([128, 1], F32)



    make_identity(nc, ident)
    # DVE probe-read of ident FIRST: all later DVE ops (and, through them,
    # ACT's constant writes) transitively imply the identity is built
    nc.vector.tensor_copy(iprobe, ident[0:1, 0:1])
    nc.vector.memset(negE2, -E2)
    nc.vector.memset(rs_a, 0.0)
    nc.vector.memset(rs_d, 0.0)
    # ones written by ACT so the final matmul waits on ACT only
    nc.scalar.activation(out=ones, in_=negE2, func=AF.Copy, bias=1.0, scale=0.0)
    nc.scalar.activation(out=dummy_bf, in_=negE2, func=AF.Copy)

    x_src = x_ap.rearrange("(c p) d -> p c d", p=128)

    # ---- fused prep + main loop, one iteration per col-group t ----
    # Each iteration preps chunk-groups 2t,2t+1 (load, norms, normalize),
    # DMA-xbar-transposes them into xnT[t], probes them on ACT (so matmuls
    # can single-wait on ACT), then runs the 8 Mtile matmuls + exps of
    # column group t. ACT's in-order queue therefore starts the first exps
    # as soon as the first two chunk-groups are ready instead of after the
    # whole prep.
    ps = ctx.enter_context(tc.tile_pool(name="ps", bufs=2, space="PSUM"))
    # kick off every input load up front: the DMA queues are idle early
    for gg in range(8):
        sl = slice(8 * gg, 8 * gg + 8)
        nc.sync.dma_start(out=x_g[gg][:, :, :], in_=x_src[:, sl, :])
    for t in range(4):
        for gg in (2 * t, 2 * t + 1):
            sl = slice(8 * gg, 8 * gg + 8)
            nc.vector.tensor_mul(
                xsq_g[gg][:, :, :], x_g[gg][:, :, :], x_g[gg][:, :, :]
            )
            nc.vector.tensor_reduce(
                out=s[:, sl],
                in_=xsq_g[gg][:, :, :],
                axis=mybir.AxisListType.X,
                op=ALU.add,
            )
            # r = s^-1/2 via exp(-0.5*ln(s)); the SQK pre-scale is folded
            # into the DVE-local copy below
            nc.scalar.activation(out=ls[:, sl], in_=s[:, sl], func=AF.Ln)
            nc.scalar.activation(
                out=r[:, sl], in_=ls[:, sl], func=AF.Exp, scale=-0.5
            )
            nc.vector.tensor_scalar_mul(
                out=r_dve[:, sl], in0=r[:, sl], scalar1=SQK
            )
            for c in range(8 * gg, 8 * gg + 8):
                nc.vector.tensor_scalar_mul(
                    out=xb[:, c, :],
                    in0=x_g[c // 8][:, c % 8, :],
                    scalar1=r_dve[:, c : c + 1],
                )
        # PE-transpose the 16 fresh chunks INTO the first main PSUM tile of
        # this group (bitcast to bf16 halves), then DVE-copy to xnT in
        # SBUF. The m=0 matmuls then overwrite the same tile, so the whole
        # chain needs no extra PSUM and every instruction carries one wait:
        # the lead dummy transpose takes the slot-WAR (ACT) merged with its
        # ACT-written input dep; real transposes wait only on DVE
        # normalize; the copy waits on PE; matmuls wait on DVE.
        pm0 = ps.tile([128, 2048], F32, tag="pm", name=f"pm_0_{t}")
        pm0_bf = pm0.bitcast(BF16)
        nc.tensor.transpose(pm0_bf[0:1, 0:128], dummy_bf, ident)
        for k in range(16):
            ch = 16 * t + k
            nc.tensor.transpose(
                pm0_bf[:, 128 * k : 128 * (k + 1)], xb[:, ch, :], ident
            )
        nc.vector.tensor_copy(xnT[t][:, :], pm0_bf[:, 0:2048])
        for m in range(8):
            j = 8 * t + m
            pm = pm0 if m == 0 else ps.tile(
                [128, 2048], F32, tag="pm", name=f"pm_{m}_{t}"
            )
            lhsT = xnT[0][:, 128 * m : 128 * (m + 1)]
            for k in range(4):
                nc.tensor.matmul(
                    pm[:, 512 * k : 512 * (k + 1)],
                    lhsT=lhsT,
                    rhs=xnT[t][:, 512 * k : 512 * (k + 1)],
                    start=True,
                    stop=True,
                )
            e_t = esc.tile([128, 2048], BF16, tag="e", name=f"e_{m}_{t}")
            nc.scalar.activation(
                out=e_t[:, :],
                in_=pm[:, :],
                func=AF.Exp,
                scale=2.0 / K16,
                accum_out=rs_a[:, j : j + 1],
            )

    # ---- positive-pair term: sum over my rows of sim(i, i+N)*K16 ----
    # local pos column of local row i is always i + 4096 (rotation invariant)
    nc.vector.tensor_mul(pos_scr, xnT[0][:, 0:RPC], xnT[2][:, 0:RPC])
    nc.vector.tensor_reduce(
        out=possum, in_=pos_scr, axis=mybir.AxisListType.X, op=ALU.add
    )

    # ---- finals: gather raw accumulators, finish (ln + sums) on host ----
    # One DVE gather makes the output DMA single-wait (DVE), and the
    # ln/partition-reduction tail (which used to cost ~10us of serial
    # ACT/DVE/PE ping-pong) moves to the host, where it is trivial.
    gather = small.tile([128, 65], F32, name="gather")
    nc.vector.tensor_copy(gather[:, 0:32], rs_a)
    nc.vector.tensor_copy(gather[:, 32:64], rs_d)
    nc.vector.tensor_copy(gather[:, 64:65], possum)
    # SWDGE for the output write: the HWDGE direct-2D encoding only
    # carries one sync wait and this DMA needs a data wait on DVE
    nc.gpsimd.dma_start(out=out_ap, in_=gather)


def _strip_waits(nc):
    """Knowledge-propagating transitive reduction of sync waits.

    Model: every sem update is attributed to its instruction in program
    order; waits gate instruction ISSUE, and each queue issues in order.
    So when instruction i on engine E waits (S >= v), E afterwards "knows"
    everything the instruction that drove S to v knew at its completion.
    A wait is redundant iff E's accumulated knowledge already implies it.

    Engine self-waits on the IMMEDIATELY PRECEDING instruction are kept
    (completion can lag issue on pipelined engines); deeper self-waits are
    dropped. Matmult/TensorScalarPtr carry only ONE wait slot in their
    encodings, so this reduction is what makes them encodable at all.
    """
    eng_prefix = {
        mybir.EngineType.PE: "PE_",
        mybir.EngineType.Activation: "Activation_",
        mybir.EngineType.DVE: "DVE_",
        mybir.EngineType.Pool: "Pool_",
        mybir.EngineType.SP: "SP_",
    }
    skip_types = {"InstEventSemaphore", "InstNotify"}

    know: dict = {e: {} for e in eng_prefix}   # engine -> {sem: floor}
    # sem -> list of (count_after_update, knowledge snapshot dict)
    snaps: dict = {}
    sem_count: dict = {}

    def merge(dst, src):
        for k, v in src.items():
            if dst.get(k, -1) < v:
                dst[k] = v

    def snap_at(sem, v):
        lst = snaps.get(sem)
        if not lst:
            return None
        for cnt, kn in lst:
            if cnt >= v:
                return kn
        return None

    for bb in nc.main_func.blocks:
        for ins in bb.instructions:
            si = ins.sync_info
            tname = type(ins).__name__
            eng = getattr(ins, "engine", None)
            if si is None or tname in skip_types or eng not in eng_prefix:
                continue
            if tname == "InstDrain":
                w = list(si.on_wait)
                if len(w) > 1 and any(
                    (x.ant_name or "").startswith("DMASW") for x in w
                ):
                    si.on_wait = [
                        x for x in w if (x.ant_name or "").startswith("DMASW")
                    ]
                continue
            pfx = eng_prefix[eng]
            K = know[eng]
            # pass 1: classify waits, gather snapshots
            cands = []
            w2 = []
            for x in list(si.on_wait):
                name = x.ant_name or ""
                try:
                    val = int(x.wait_value)
                except (TypeError, ValueError):
                    w2.append(x)
                    continue
                if name.startswith(pfx):
                    # self-wait: queue order implies it, except possibly for
                    # the immediately preceding instruction on a pipelined
                    # engine. Keep that case (except matmuls: PE serializes)
                    if (
                        val >= sem_count.get(name, 0)
                        and tname != "InstMatmult"
                    ):
                        w2.append(x)
                    continue
                cands.append((x, name, val, snap_at(name, val)))
            # pass 2: drop waits implied by prior knowledge or by the
            # knowledge carried by the OTHER waits' snapshots
            for i, (x, name, val, sn) in enumerate(cands):
                implied = K.get(name, -1) >= val
                if not implied:
                    for j2, (_, n2, v2, sn2) in enumerate(cands):
                        if j2 == i:
                            continue
                        if n2 == name and v2 >= val:
                            implied = i > j2  # keep one of equal waits
                        if sn2 and sn2.get(name, -1) >= val:
                            implied = True
                        if implied:
                            break
                if not implied:
                    w2.append(x)
                K[name] = max(K.get(name, 0), val)
                if sn:
                    merge(K, sn)
            si.on_wait = w2
            # record this instruction's updates with a knowledge snapshot
            for u in list(si.on_update or []):
                name = u.ant_name or ""
                try:
                    uval = int(u.update_value)
                except (TypeError, ValueError):
                    uval = 1
                c = sem_count.get(name, 0) + uval
                sem_count[name] = c
                K[name] = max(K.get(name, 0), c)
                snaps.setdefault(name, []).append((c, dict(K)))


def _build(strip: bool = True):
    from contextlib import ExitStack

    nc = bass.Bass("TRN2", debug=False, num_devices=NCORES)
    x_in = nc.dram_tensor("x", [N2, D], F32, kind="ExternalInput")
    out = nc.dram_tensor("out", [128, 65], F32, kind="ExternalOutput")
    dbg_aps = None
    if DEBUG_OUTS:
        shapes = {"s": [128, 64], "r": [128, 64], "rs_a": [128, 32],
                  "rs_d": [128, 32], "rt": [128, 8], "lg": [128, 8],
                  "xb": [128, 128], "xnt": [128, 128]}
        dbg_aps = {
            k: nc.dram_tensor(f"dbg_{k}", v, F32 if k not in ("xb", "xnt")
                              else mybir.dt.bfloat16,
                              kind="ExternalOutput").ap()
            for k, v in shapes.items()
        }
    with tile.TileContext(nc) as tc:
        with ExitStack() as ctx:
            _emit(tc, ctx, out.ap(), x_in.ap(), dbg_aps)
    if strip:
        # CoreSim's race detector models engines as concurrent and would
        # flag the removed (redundant-on-HW) waits; validate numerics
        # with strip=False, ship with strip=True.
        _strip_waits(nc)
    return nc


_NC_CACHE = None


def _get_nc():
    global _NC_CACHE
    if _NC_CACHE is None:
        _NC_CACHE = _build()
    return _NC_CACHE


def kernel(**inputs) -> np.ndarray:
    x = np.ascontiguousarray(
        np.asarray(inputs["projected_vectors"]), dtype=np.float32
    )
    assert x.shape == (N2, D)
    nc = _get_nc()
    in_maps = [
        {"x": np.ascontiguousarray(np.roll(x, -RPC * c, axis=0))}
        for c in range(NCORES)
    ]
    res = run_bass_kernel_spmd(nc, in_maps, core_ids=list(range(NCORES)))
    return finish(res.results)


def finish(results) -> np.ndarray:
    """Host-side finale: per-core ln(rowsum - e^2) sums + positive terms."""
    total = 0.0
    for rmap in results:
        g = np.asarray(rmap["out"], dtype=np.float64)
        rs = g[:, 0:32] + g[:, 32:64]           # [p, 8t+m] partial row sums
        rowsum = rs.reshape(128, 4, 8).sum(axis=1)   # [p, m]
        possum = g[:, 64].sum()                 # sum over partitions (dims)
        total += np.sum(np.log(rowsum - E2)) - (2.0 / K16) * possum
    return np.asarray(total, dtype=np.float32)


if __name__ == "__main__":
    xt = np.random.randn(N2, D).astype(np.float32)
    print(kernel(projected_vectors=xt))
